# revision 49
# baseline (speedup 1.0000x reference)
"""Trainium2 Bass kernel for Mistral-style MHA prefill (sparse_attention).

Problem: B=2, S=2048, DIM=4096, 32 q heads / 8 kv heads, head_dim=128,
sliding window 2048 (== S, so the mask is pure causal), RoPE, fp32 reference.

Sharding (8 cores): data-parallel over batch (2) x tensor-parallel over heads
(4).  Core c = b*4 + tp handles batch b, q-heads [tp*8, tp*8+8), kv-heads
[tp*2, tp*2+2).  wq/wk/wv are sharded column-wise (output dim), wo row-wise
(input dim); the all-reduce after wo is done on the host (numpy sum of the 4
fp32 partials per batch).

On-chip dataflow per core (all matmul inputs bf16, accumulation fp32), built
around a transpose-free attention pipeline:
  - The sequence is processed in 4 quarters of 512.  Per quarter: x^T chunk
    DMA, V projection (natural [s, dh+ones]), K projection + RoPE ([dh, s]),
    Q projection + RoPE ([dh, s]) for all 8 local heads.
  - Scores are computed TRANSPOSED: S^T[k, q] = K^T-block.T @ Q^T-block, so
    the exp output P^T[k, q] is directly the stationary operand of the PV
    matmul - no 128x128 PE transposes of P (the baseline spent ~8% of PE
    time there).  The softmax row-sum comes for free from a ones-column
    appended to V: PV emits A[q, dh] with the row-sum in column dh, and the
    per-partition (per-q) reciprocal normalizes during the PSUM->SBUF copy.
  - wo: A blocks PE-transposed to A^T, out[s,e] += A^T-chunk.T @ wo^T chunk.
    The wo work for q-block qi-1 is interleaved chunk-by-chunk between the
    attention heads of q-block qi, giving the PE filler work while the
    scalar engine (exp) is the per-head rate limiter.
"""

import os
import sys

import numpy as np

for _p in ("/opt/trn_rl_repo",):
    if _p not in sys.path and os.path.isdir(_p):
        sys.path.insert(0, _p)

import ml_dtypes  # noqa: E402

import concourse.bass as bass  # noqa: E402
import concourse.mybir as mybir  # noqa: E402
import concourse.tile as tile  # noqa: E402
from concourse.bass_utils import run_bass_kernel_spmd  # noqa: E402

BF16 = ml_dtypes.bfloat16


def _install_drain_split_patch():
    """The pinned walrus rejects Tile's kernel-tail Drain when it carries more
    than ~2 semaphore waits ("Too many sync wait commands").  Split the global
    drain's waits across trailing sync-engine nops (1 wait each); all waits
    still complete before the all-engine barrier and semaphore reset."""
    if getattr(tile.TileContext, "_drain_split_patched", False):
        return
    from concourse.vector_clock import ScopedClock

    limit = 2

    def _patched_dab(self, tick_clock, wait_clock):
        drain_inst = self.nc.sync.drain()
        raw = drain_inst.ins
        wait_clock.add_sem_waits(raw, ScopedClock({None: tick_clock.global_clock}))
        si = raw.sync_info
        waits = list(si.on_wait or [])
        if len(waits) > limit:
            si.on_wait = waits[:limit]
            for i in range(limit, len(waits), limit):
                nraw = self.nc.sync.nop().ins
                nsi = nraw.sync_info
                if nsi is None:
                    nraw.sync_info = mybir.SyncInfo(
                        on_wait=waits[i : i + limit], on_update=[]
                    )
                else:
                    nsi.on_wait = list(nsi.on_wait or []) + waits[i : i + limit]
        self.nc.all_engine_barrier()
        popped = self.nc._tile_sem_poison_stack.pop()
        assert popped is self._sem_poison
        self.nc.clear_and_free_semaphores(list(self.sems.allocated().values()))
        self.nc.all_engine_barrier()

    tile.TileContext._drain_and_barrier = _patched_dab
    tile.TileContext._drain_split_patched = True


_install_drain_split_patch()

P = 128
S = 2048
D = 4096
KO = D // P  # 32 contraction chunks
SQ = S // 4  # quarter of the sequence (512)
NQ = 4  # quarters
NB = S // P  # 16 q/k blocks of 128
NH_L = 8  # q heads per core
NKV_L = 2  # kv heads per core
DH = 128
SCALE = float(DH) ** -0.5
N_CORES = 8

_dt_f32 = mybir.dt.float32
_dt_bf16 = mybir.dt.bfloat16


def _emit(tc, aps):
    nc = tc.nc
    # All weight/x chunks are host-relaid so each DMA has one contiguous
    # multi-KB run per partition (see make_in_maps).
    xr = aps["xT"].rearrange("(c p) s -> p c s", p=P)  # [128, 64, 1024]
    wqr = aps["wqT"].rearrange("(c p) o -> p c o", p=P)  # [128, 8, 4096]
    wkvr = aps["wkvT"].rearrange("(c p) o -> p c o", p=P)  # [128, 4, 4096]
    wor = aps["woT"].rearrange("(c p) e -> p c e", p=P)  # [128, 4, 8192]
    out_ap = aps["out"]  # [2048, 4096] f32

    from contextlib import ExitStack

    with ExitStack() as g:
        singles = g.enter_context(tc.tile_pool(name="singles", bufs=1))
        small = g.enter_context(tc.tile_pool(name="small", bufs=6))
        kv_pool = g.enter_context(tc.tile_pool(name="kv", bufs=1))
        wo_pool = g.enter_context(tc.tile_pool(name="wop", bufs=1))
        xt_pool = g.enter_context(tc.tile_pool(name="xt", bufs=1))
        wstage = g.enter_context(tc.tile_pool(name="wstage", bufs=2))
        qt_pool = g.enter_context(tc.tile_pool(name="qt", bufs=1))
        a_pool = g.enter_context(tc.tile_pool(name="ap", bufs=3))
        at_pool = g.enter_context(tc.tile_pool(name="atp", bufs=2))
        pt_pool = g.enter_context(tc.tile_pool(name="ptp", bufs=5))
        rope_pool = g.enter_context(tc.tile_pool(name="rope", bufs=2))
        ostage = g.enter_context(tc.tile_pool(name="ostage", bufs=6))
        ps_mm = g.enter_context(tc.tile_pool(name="ps_mm", bufs=3, space="PSUM"))
        ps_sc = g.enter_context(tc.tile_pool(name="ps_sc", bufs=3, space="PSUM"))
        ps_av = g.enter_context(tc.tile_pool(name="ps_av", bufs=1, space="PSUM"))
        ps_tp = g.enter_context(tc.tile_pool(name="ps_tp", bufs=1, space="PSUM"))

        # tiles declared here; their DMAs are issued inside quarter 0 after
        # the first x/weight chunks (they are not needed until the first
        # rope / attention, ~25us in, and would otherwise delay the
        # startup-critical V-projection feed).
        cexp_t = singles.tile([P, S], _dt_bf16)
        sexp_t = singles.tile([P, S], _dt_bf16)
        perm_t = singles.tile([P, P], _dt_bf16)
        ident_t = singles.tile([P, P], _dt_bf16)
        maskT_t = singles.tile([P, P], _dt_f32)

        # K^T (roped) for the whole sequence, V natural with a ones column
        # per kv head ([k, dh+1] slices for the PV rowsum trick).
        kt_t = kv_pool.tile([P, NKV_L, S], _dt_bf16)
        v_t = kv_pool.tile([P, NB, NKV_L, DH + 1], _dt_bf16)
        nc.vector.memset(v_t[:, :, :, DH : DH + 1], 1.0)

        # wo weights, loaded once (needed from the first wo block onward).
        wo_t = wo_pool.tile([P, NH_L, D], _dt_bf16)

        cp_flip = [0]

        def cp(out, in_):
            # Alternate PSUM->SBUF copies between the scalar and vector engines.
            if cp_flip[0] % 2 == 0:
                nc.scalar.copy(out=out, in_=in_)
            else:
                nc.vector.tensor_copy(out=out, in_=in_)
            cp_flip[0] += 1

        def rope_finish(dst, s_off):
            """Pair-swap matmul + cos/sin combine; the scalar copy of dst
            happened earlier, so the perm matmul's input is long ready by the
            time the PE reaches it (software-pipelined past the next proj)."""
            sw = ps_mm.tile([P, SQ], _dt_f32, tag="mm")
            nc.tensor.matmul(sw, lhsT=perm_t, rhs=dst, start=True, stop=True)
            t1 = rope_pool.tile([P, SQ], _dt_bf16, tag="t1")
            nc.vector.tensor_mul(t1, sw, sexp_t[:, s_off : s_off + SQ])
            nc.vector.tensor_mul(dst, dst, cexp_t[:, s_off : s_off + SQ])
            nc.vector.tensor_add(dst, dst, t1)

        # ---------------- wo chunk machinery (delayed by one q-block) -------
        def make_wo_chunks(qi, a_b):
            """Closures emitting the wo projection of q-block qi from a_b
            (normalized A, [128, 8*128] bf16).  ~10 chunks of PE work."""
            at = at_pool.tile([P, NH_L, P], _dt_bf16, tag="at")

            def tchunk(hb4):
                def f():
                    tp4 = ps_tp.tile([P, 4, P], _dt_bf16, tag="tp4")
                    for j in range(4):
                        nc.tensor.transpose(
                            tp4[:, j, :],
                            a_b[:, (hb4 + j) * P : (hb4 + j + 1) * P],
                            ident_t,
                        )
                    cp(at[:, hb4 : hb4 + 4, :], tp4)

                return f

            def echunk(ec):
                def f():
                    go = ps_mm.tile([P, 512], _dt_f32, tag="mm")
                    for hb in range(NH_L):
                        nc.tensor.matmul(
                            go,
                            lhsT=at[:, hb, :],
                            rhs=wo_t[:, hb, ec * 512 : (ec + 1) * 512],
                            start=(hb == 0),
                            stop=(hb == NH_L - 1),
                        )
                    ost = ostage.tile([P, 512], _dt_bf16, tag="ost")
                    cp(ost, go)
                    nc.sync.dma_start(
                        out=out_ap[qi * P : (qi + 1) * P, ec * 512 : (ec + 1) * 512],
                        in_=ost,
                    )

                return f

            return [tchunk(0), tchunk(4)] + [echunk(ec) for ec in range(8)]

        # ---------------- attention for one (head, q-block) -----------------
        def attn_head(h, qi, qt, qil, a_b):
            """S^T -> exp -> PV(+rowsum) -> normalized A block into a_b."""
            g2 = h // 4
            nblk = qi + 1
            ngrp = (nblk + 3) // 4
            qblk = qt[:, h, qil * P : (qil + 1) * P]
            pts = []
            for gi in range(ngrp):
                k0 = gi * 4
                nj = min(4, nblk - k0)
                ssc = ps_sc.tile([P, 4, P], _dt_f32, tag="sc")
                for j in range(nj):
                    kb = k0 + j
                    nc.tensor.matmul(
                        ssc[:, j, :],
                        lhsT=kt_t[:, g2, kb * P : (kb + 1) * P],
                        rhs=qblk,
                        start=True,
                        stop=True,
                    )
                if k0 + nj == nblk:  # group holds the diagonal block
                    nc.vector.tensor_add(
                        ssc[:, nj - 1, :], ssc[:, nj - 1, :], maskT_t
                    )
                pt = pt_pool.tile([P, 4, P], _dt_bf16, tag="pt")
                nc.scalar.activation(
                    out=pt[:, :nj, :],
                    in_=ssc[:, :nj, :],
                    func=mybir.ActivationFunctionType.Exp,
                    scale=SCALE,
                )
                pts.append((pt, k0, nj))
            return (pts, g2, nblk, h, a_b)

        def attn_head_pv(state):
            pts, g2, nblk, h, a_b = state
            pa = ps_av.tile([P, DH + 4], _dt_f32, tag="pa")
            for pt, k0, nj in pts:
                for j in range(nj):
                    kb = k0 + j
                    nc.tensor.matmul(
                        pa[:, : DH + 1],
                        lhsT=pt[:, j, :],
                        rhs=v_t[:, kb, g2, :],
                        start=(kb == 0),
                        stop=(kb == nblk - 1),
                    )
            rinv = small.tile([P, 1], _dt_f32, tag="rinv")
            nc.vector.reciprocal(rinv, pa[:, DH : DH + 1])
            nc.vector.tensor_scalar_mul(
                a_b[:, h * P : (h + 1) * P], pa[:, :DH], rinv
            )

        # ---------------- main loop over sequence quarters ------------------
        pending_wo = []  # chunks of the delayed wo block

        for Q in range(NQ):
            s0 = Q * SQ
            # V weights first (first consumer), then the x^T chunk for this
            # quarter, s-sub-block-major so V proj of the first 128 rows can
            # start after ~3MB of efficient (contiguous-run) DMA.
            wkv_v = wstage.tile([P, KO, 256], _dt_bf16, tag="w")
            nc.sync.dma_start(out=wkv_v[:, 0:16, :], in_=wkvr[:, 2, :])
            # xt is s-sub-block-major: [p, sblk, ko, s128]; V proj of sb0 can
            # start on ko 0..15 after just wkv chunk0 + xt sblk0 (2MB).
            xt = xt_pool.tile([P, 4, KO, P], _dt_bf16, tag="xt")
            for kq in range(4):
                nc.sync.dma_start(
                    out=xt[:, 0, kq * 8 : (kq + 1) * 8, :],
                    in_=xr[:, Q * 16 + kq, :],
                )
            nc.sync.dma_start(out=wkv_v[:, 16:32, :], in_=wkvr[:, 3, :])
            for sblk in range(1, 4):
                for kq in range(4):
                    nc.sync.dma_start(
                        out=xt[:, sblk, kq * 8 : (kq + 1) * 8, :],
                        in_=xr[:, Q * 16 + sblk * 4 + kq, :],
                    )

            pending_rope = []  # deferred rope tails (perm matmul + DVE)

            def pop_rope():
                if pending_rope:
                    pending_rope.pop(0)()

            # ---- V projection (natural [s, dh] + ones cols preserved) ----
            for sbl in range(4):
                sb = Q * 4 + sbl
                gv = ps_mm.tile([P, 512], _dt_f32, tag="mm")
                for ko in range(KO):
                    nc.tensor.matmul(
                        gv[:, :256],
                        lhsT=xt[:, sbl, ko, :],
                        rhs=wkv_v[:, ko, :],
                        start=(ko == 0),
                        stop=(ko == KO - 1),
                    )
                # gv holds [s, 2*128]; scatter to the two dh slots (stride 129)
                cp(
                    v_t[:, sb, :, 0:DH],
                    gv[:, 0:256],
                )

            # ---- K projection + rope ([dh, s] layout) ----
            wkv_k = wstage.tile([P, KO, 256], _dt_bf16, tag="w")
            for i in range(2):
                nc.sync.dma_start(
                    out=wkv_k[:, i * 16 : (i + 1) * 16, :], in_=wkvr[:, 0 + i, :]
                )
            if Q == 0:
                nc.sync.dma_start(out=cexp_t, in_=aps["cexp"])
                nc.sync.dma_start(out=sexp_t, in_=aps["sexp"])
                nc.sync.dma_start(out=perm_t, in_=aps["perm"])
                nc.sync.dma_start(out=ident_t, in_=aps["ident"])
                nc.sync.dma_start(out=maskT_t, in_=aps["maskT"])
            for g2 in range(NKV_L):
                gk = ps_mm.tile([P, SQ], _dt_f32, tag="mm")
                for ko in range(KO):
                    nc.tensor.matmul(
                        gk,
                        lhsT=wkv_k[:, ko, g2 * P : (g2 + 1) * P],
                        rhs=xt[:, :, ko, :],
                        start=(ko == 0),
                        stop=(ko == KO - 1),
                    )
                dst = kt_t[:, g2, s0 : s0 + SQ]
                nc.scalar.copy(out=dst, in_=gk)
                pending_rope.append(lambda d=dst, s=s0: rope_finish(d, s))
                if g2 == 1:
                    pop_rope()

            # ---- Q projection + rope for all 8 local heads ----
            qt = qt_pool.tile([P, NH_L, SQ], _dt_bf16, tag="qt")
            for hp in range(4):
                wq_hp = wstage.tile([P, KO, 256], _dt_bf16, tag="w")
                for i in range(2):
                    nc.sync.dma_start(
                        out=wq_hp[:, i * 16 : (i + 1) * 16, :],
                        in_=wqr[:, hp * 2 + i, :],
                    )
                for h2 in range(2):
                    gq = ps_mm.tile([P, SQ], _dt_f32, tag="mm")
                    for ko in range(KO):
                        nc.tensor.matmul(
                            gq,
                            lhsT=wq_hp[:, ko, h2 * P : (h2 + 1) * P],
                            rhs=xt[:, :, ko, :],
                            start=(ko == 0),
                            stop=(ko == KO - 1),
                        )
                    dst = qt[:, hp * 2 + h2, :]
                    nc.scalar.copy(out=dst, in_=gq)
                    pending_rope.append(lambda d=dst, s=s0: rope_finish(d, s))
                    pop_rope()
            # the last head's rope tail is emitted inside the attention loop
            # below (after the first S-group) so its perm matmul never waits
            # on the scalar copy.

            if Q == 0:
                # wo weights: issued after the first quarter's weight DMAs so
                # they don't delay the pipeline start; first consumed by the
                # qi=0 wo chunks that run interleaved with attn(qi=1).
                for i in range(4):
                    nc.sync.dma_start(
                        out=wo_t[:, i * 2 : (i + 1) * 2, :], in_=wor[:, i, :]
                    )

            # ---- attention + interleaved (delayed) wo ----
            for qil in range(4):
                qi = Q * 4 + qil
                a_b = a_pool.tile([P, NH_L * DH], _dt_bf16, tag="ab")
                # Pace the previous block's wo chunks across this block's
                # heads: the chunks are pure-PE filler while the scalar
                # engine works through the exps.
                for h in range(NH_L):
                    st = attn_head(h, qi, qt, qil, a_b)
                    # proportional pacing: consume all delayed-wo chunks by
                    # the last head so the next wo block starts immediately
                    npop = -(-len(pending_wo) // (NH_L - h))
                    if h < 2:
                        npop = min(npop, 1)
                    for _ in range(npop):
                        pending_wo.pop(0)()
                    pop_rope()
                    attn_head_pv(st)
                while pending_wo:
                    pending_wo.pop(0)()
                pending_wo = make_wo_chunks(qi, a_b)

        # final wo block (qi = 15)
        while pending_wo:
            pending_wo.pop(0)()


def _split_excess_waits(nc, limit=1):
    """Walrus (pinned build) rejects instructions carrying more than ~2
    semaphore waits.  Hoist excess waits onto same-engine no-ops inserted
    immediately before the offending instruction: the engine executes the
    nop's waits first, so the AND-semantics of the wait set is preserved."""
    ctr = [0]
    for bb in nc.main_func.blocks:
        insts = list(bb.instructions)
        out = []
        changed = False
        for ins in insts:
            si = ins.sync_info
            waits = list(si.on_wait) if si and si.on_wait else []
            if len(waits) > limit:
                keep = waits[:limit]
                rest = waits[limit:]
                for i in range(0, len(rest), limit):
                    nop = mybir.InstNoOp(name=f"I-waitsplit-{ctr[0]}", ins=[], outs=[])
                    ctr[0] += 1
                    nop.engine = ins.engine
                    nop.sync_info = mybir.SyncInfo(
                        on_wait=rest[i : i + limit], on_update=[]
                    )
                    nc.register_instruction(nop)
                    out.append(nop)
                si.on_wait = keep
                changed = True
            out.append(ins)
        if changed:
            bb.instructions = out
    return ctr[0]


_PROGRAM_CACHE = {}


def build_program():
    if "nc" in _PROGRAM_CACHE:
        return _PROGRAM_CACHE["nc"]
    nc = bass.Bass("TRN2", target_bir_lowering=False, debug=False, num_devices=N_CORES)
    aps = {
        # chunk-major relayouts: row (c, p) holds one contiguous per-partition
        # run of the DMA chunk c (see make_in_maps)
        "xT": nc.dram_tensor("xT", [64 * P, 8 * P], _dt_bf16, kind="ExternalInput").ap(),
        "wqT": nc.dram_tensor("wqT", [8 * P, 16 * 256], _dt_bf16, kind="ExternalInput").ap(),
        "wkvT": nc.dram_tensor("wkvT", [4 * P, 16 * 256], _dt_bf16, kind="ExternalInput").ap(),
        "woT": nc.dram_tensor("woT", [4 * P, 2 * D], _dt_bf16, kind="ExternalInput").ap(),
        "cexp": nc.dram_tensor("cexp", [P, S], _dt_bf16, kind="ExternalInput").ap(),
        "sexp": nc.dram_tensor("sexp", [P, S], _dt_bf16, kind="ExternalInput").ap(),
        "perm": nc.dram_tensor("perm", [P, P], _dt_bf16, kind="ExternalInput").ap(),
        "ident": nc.dram_tensor("ident", [P, P], _dt_bf16, kind="ExternalInput").ap(),
        "maskT": nc.dram_tensor("maskT", [P, P], _dt_f32, kind="ExternalInput").ap(),
        "out": nc.dram_tensor("out", [S, D], _dt_bf16, kind="ExternalOutput").ap(),
    }
    with tile.TileContext(nc) as tc:
        _emit(tc, aps)
    _split_excess_waits(nc, limit=1)
    _PROGRAM_CACHE["nc"] = nc
    return nc


def make_in_maps(x, freqs_cos, freqs_sin, mask, wq, wk, wv, wo):
    x = np.asarray(x, np.float32)
    freqs_cos = np.asarray(freqs_cos, np.float32)
    freqs_sin = np.asarray(freqs_sin, np.float32)
    mask = np.asarray(mask, np.float32)
    wq = np.asarray(wq, np.float32)
    wk = np.asarray(wk, np.float32)
    wv = np.asarray(wv, np.float32)
    wo = np.asarray(wo, np.float32)

    def relayout_x(xT):
        # [4096, 2048] -> chunk-major [64*128, 1024]; chunk c=(Q,sblk,kq)
        # holds [p, ko8, s128] with a contiguous 2KB run per partition.
        a = xT.reshape(4, 8, P, 4, 4, P)  # [kq, k8, p, Q, sb, s]
        return np.ascontiguousarray(
            a.transpose(3, 4, 0, 2, 1, 5).reshape(64 * P, 8 * P)
        )

    def relayout_w(wT, nchunk):
        # [4096, nchunk*256] -> [(nchunk*2)*128, 16*256]; chunk (j, half)
        # holds [p, ko16, o256] with a contiguous 8KB run per partition.
        a = wT.reshape(2, 16, P, nchunk, 256)  # [half, k16, p, j, o]
        return np.ascontiguousarray(
            a.transpose(3, 0, 2, 1, 4).reshape(nchunk * 2 * P, 16 * 256)
        )

    xb = [relayout_x(x[b].T.astype(BF16)) for b in range(2)]
    cexp = np.repeat(freqs_cos.T, 2, axis=0).astype(BF16)  # (128, 2048)
    sx = np.repeat(freqs_sin.T, 2, axis=0).astype(np.float32)
    sx[0::2] *= -1.0
    sexp = sx.astype(BF16)
    perm = np.zeros((P, P), np.float32)
    idx = np.arange(P)
    perm[idx, idx ^ 1] = 1.0
    perm = perm.astype(BF16)
    ident = np.eye(P, dtype=np.float32).astype(BF16)
    maskT = np.ascontiguousarray(mask[:P, :P].T, dtype=np.float32)

    in_maps = []
    for core in range(N_CORES):
        b, tp = core // 4, core % 4
        wqT = relayout_w(
            wq[tp * 1024 : (tp + 1) * 1024].T.astype(BF16), 4
        )  # (4096, 1024) -> chunks hp0..3
        wkT = wk[tp * 256 : (tp + 1) * 256].T.astype(BF16)  # (4096, 256)
        wvT = wv[tp * 256 : (tp + 1) * 256].T.astype(BF16)
        wkvT = relayout_w(
            np.concatenate([wkT, wvT], axis=1), 2
        )  # chunk 0 = K, chunk 1 = V
        woTn = wo[:, tp * 1024 : (tp + 1) * 1024].T.astype(BF16)  # (1024, 4096)
        woT = np.ascontiguousarray(
            woTn.reshape(4, 2, P, D).transpose(0, 2, 1, 3).reshape(4 * P, 2 * D)
        )
        in_maps.append(
            {
                "xT": xb[b],
                "wqT": wqT,
                "wkvT": wkvT,
                "woT": woT,
                "cexp": cexp,
                "sexp": sexp,
                "perm": perm,
                "ident": ident,
                "maskT": maskT,
            }
        )
    return in_maps


def run(inputs, trace=False):
    nc = build_program()
    in_maps = make_in_maps(
        inputs["x"],
        inputs["freqs_cos"],
        inputs["freqs_sin"],
        inputs["mask"],
        inputs["wq"],
        inputs["wk"],
        inputs["wv"],
        inputs["wo"],
    )
    res = run_bass_kernel_spmd(nc, in_maps, list(range(N_CORES)), trace=trace)
    out = np.zeros((2, S, D), np.float32)
    for core in range(N_CORES):
        out[core // 4] += np.asarray(res.results[core]["out"], np.float32)
    return out, res


def kernel(x, freqs_cos, freqs_sin, positions, mask, wq, wk, wv, wo):
    out, _ = run(
        {
            "x": x,
            "freqs_cos": freqs_cos,
            "freqs_sin": freqs_sin,
            "mask": mask,
            "wq": wq,
            "wk": wk,
            "wv": wv,
            "wo": wo,
        }
    )
    return out
